# revision 24
# baseline (speedup 1.0000x reference)
"""AdaptiveTokenSampling kernel for 8 TRN2 NeuronCores.

Data-parallel over batch: core i handles batch element i end-to-end
(scoring, gumbel top-k argmax, dedup+compact, attention-row gather).

Pipeline per core (b = batch element):
  s[t]     = sum_h attn[b,h,0,t] * ||value[b,h,t,:]||        (t = 1..1023)
  logits   = log(s / (sum_t s + eps) + eps), masked
  sampled1..256 = argmax_t(logits[t] + gumbel[b,k,t-1])      (256 draws)
  uniq     = sorted unique sampled ids, 0-padded, cls(0) prepended
  new_attn[h,s,:] = attn[b,h,uniq[s],:]

Dedup is sort-free: a histogram of sampled ids is built with two
separable-equality matmuls, ranks come from a triangular-matrix prefix-sum
matmul, and the compacted ids from an equality matmul against the ranks.

Layouts: "L1" token layout t = 8p + j ([128, H*8] tiles, DMA-friendly),
"chunk" layout t = 128c + p ([128, 8] tiles, matmul-friendly); the one
L1->row hop is a PE transpose + affine_select block-diagonal broadcast.
"""

import sys

for p in ("/opt/trn_rl_repo", "/root/.axon_site/_ro/trn_rl_repo"):
    if p not in sys.path:
        sys.path.insert(0, p)

import numpy as np

import concourse.bass as bass
import concourse.mybir as mybir
from concourse.tile import TileContext

B, H, N, D = 8, 12, 1024, 64
K = 256          # number of gumbel draws
S = K + 1        # output slots (cls prepended)
EPS = 1e-6
MASK_VALUE = -np.finfo(np.float32).max / 2
NEG_BIG = -1.0e30

F32 = mybir.dt.float32
F16 = mybir.dt.float16
I32 = mybir.dt.int32
U32 = mybir.dt.uint32
U8 = mybir.dt.uint8
ALU = mybir.AluOpType
ACTF = mybir.ActivationFunctionType


def split_multi_waits(nc, max_waits=1):
    """Walrus codegen rejects instructions with several sem waits; hoist
    extra waits onto same-engine NoOps inserted immediately before."""
    for fn in nc.m.functions:
        for blk in fn.blocks:
            new_insts = []
            for inst in blk.instructions:
                si = getattr(inst, "sync_info", None)
                if si is not None and si.on_wait and len(si.on_wait) > max_waits:
                    waits = list(si.on_wait)
                    for j, w in enumerate(waits[:-max_waits]):
                        new_insts.append(
                            mybir.InstNoOp(
                                name=f"{inst.name}_wsplit{j}",
                                sync_info=mybir.SyncInfo(on_wait=[w], on_update=[]),
                                bass_nofuse=True,
                                engine=inst.engine,
                            )
                        )
                    inst.sync_info = mybir.SyncInfo(
                        on_wait=waits[-max_waits:], on_update=si.on_update
                    )
                new_insts.append(inst)
            blk.instructions[:] = new_insts


def build_nc(waitsplit=True):
    nc = bass.Bass()

    attn_in = nc.declare_dram_parameter("attn", [H * N, N], F32, isOutput=False)
    value_in = nc.declare_dram_parameter("value", [H * N, D], F32, isOutput=False)
    mask_in = nc.declare_dram_parameter("mask", [N], U8, isOutput=False)
    gumbel_in = nc.declare_dram_parameter("gumbel", [K, N - 1], F32, isOutput=False)

    out_attn = nc.declare_dram_parameter("out_attn", [H, S * N], F32, isOutput=True)
    out_uniq = nc.declare_dram_parameter("out_uniq", [S], I32, isOutput=True)
    out_mask = nc.declare_dram_parameter("out_mask", [S], U8, isOutput=True)

    VC = 6               # value DMA/norm pipeline chunks
    HC = H // VC         # heads per chunk

    with TileContext(nc) as tc:
        with (
            tc.tile_pool(name="const", bufs=1) as cpool,
            tc.tile_pool(name="sbuf", bufs=1) as pool,
            tc.tile_pool(name="gather", bufs=8) as gpool,
            tc.tile_pool(name="psumA", bufs=2, space="PSUM") as ppoolA,
            tc.tile_pool(name="psumB", bufs=3, space="PSUM") as ppoolB,
        ):
            # ---------------- input DMAs first ----------------
            # value in "L1" layout: v_sb[p, h*512 + j*64 + d] = value[h, 8p+j, d]
            v_sb = pool.tile([128, H * 8 * D], F32, tag="v_sb")
            v_dst = v_sb[:].rearrange("p (h j d) -> p h j d", h=H, j=8, d=D)
            v_src = value_in[:].rearrange("(h p j) d -> p h j d", h=H, p=128, j=8)
            for m in range(VC):
                nc.sync.dma_start(
                    out=v_dst[:, m * HC : (m + 1) * HC],
                    in_=v_src[:, m * HC : (m + 1) * HC],
                )

            # gumbel tiles: g_sb[k2][p, t] = gumbel[k2*128+p, t-1]; col 0 = 0
            g_sb = []
            for k2 in range(2):
                g = pool.tile([128, N], F32, tag=f"g_sb{k2}")
                nc.sync.dma_start(
                    out=g[:, 1:N], in_=gumbel_in[k2 * 128 : (k2 + 1) * 128, :]
                )
                nc.vector.memset(g[:, 0:1], 0.0)
                g_sb.append(g)

            # cls attention rows in L1: ca[p, h*8+j] = attn[h*N, 8p+j]
            ca = pool.tile([128, H * 8], F32, tag="ca")
            nc.sync.dma_start(
                out=ca[:].rearrange("p (h j) -> p h j", h=H),
                in_=attn_in[:].rearrange("(h n) (p j) -> n p h j", h=H, p=128)[0],
            )

            # mask in L1: mask_l1[p, j] = mask[8p+j]
            mask_l1 = pool.tile([128, 8], U8, tag="mask_l1")
            nc.sync.dma_start(
                out=mask_l1[:], in_=mask_in[:].rearrange("(p j) -> p j", p=128)
            )

            # ---------------- constants ----------------
            iota_p_i = cpool.tile([128, 128], I32, tag="iota_p_i")
            nc.gpsimd.iota(iota_p_i[:], pattern=[[1, 128]], channel_multiplier=0)
            iota_p_row = cpool.tile([128, 128], F32, tag="iota_p_row")
            nc.vector.tensor_copy(iota_p_row[:], iota_p_i[:])

            iota_c_i = cpool.tile([128, 8], I32, tag="iota_c_i")
            nc.gpsimd.iota(iota_c_i[:], pattern=[[1, 8]], channel_multiplier=0)
            iota_c_row = cpool.tile([128, 8], F32, tag="iota_c_row")
            nc.vector.tensor_copy(iota_c_row[:], iota_c_i[:])

            iota_tok_i = cpool.tile([128, 8], I32, tag="iota_tok_i")
            nc.gpsimd.iota(iota_tok_i[:], pattern=[[128, 8]], channel_multiplier=1)
            iota_tok_h = cpool.tile([128, 8], F16, tag="iota_tok_h")
            nc.vector.tensor_copy(iota_tok_h[:], iota_tok_i[:])

            tri_i = cpool.tile([128, 128], I32, tag="tri_i")
            nc.gpsimd.iota(tri_i[:], pattern=[[1, 128]], channel_multiplier=-1)
            tri_raw = cpool.tile([128, 128], F32, tag="tri_raw")
            nc.vector.tensor_copy(tri_raw[:], tri_i[:])
            tri_h = cpool.tile([128, 128], F16, tag="tri_h")
            nc.vector.tensor_scalar(
                out=tri_h[:], in0=tri_raw[:], scalar1=0.0, scalar2=None, op0=ALU.is_ge
            )

            iota_s_i = cpool.tile([128, S], I32, tag="iota_s_i")
            nc.gpsimd.iota(iota_s_i[:], pattern=[[1, S]], channel_multiplier=0)
            iota_s_f = cpool.tile([128, S], F32, tag="iota_s_f")
            nc.vector.tensor_copy(iota_s_f[:], iota_s_i[:])

            iota_h_i = cpool.tile([12, 1], I32, tag="iota_h_i")
            nc.gpsimd.iota(iota_h_i[:], pattern=[[0, 1]], channel_multiplier=N)
            iota_h_f = cpool.tile([12, 1], F32, tag="iota_h_f")
            nc.vector.tensor_copy(iota_h_f[:], iota_h_i[:])

            iota_pc_i = cpool.tile([128, 1], I32, tag="iota_pc_i")
            nc.gpsimd.iota(iota_pc_i[:], pattern=[[0, 1]], channel_multiplier=1)
            iota_pc_f = cpool.tile([128, 1], F32, tag="iota_pc_f")
            nc.vector.tensor_copy(iota_pc_f[:], iota_pc_i[:])
            ident = cpool.tile([128, 128], F32, tag="ident")
            nc.vector.tensor_scalar(
                out=ident[:], in0=iota_p_row[:], scalar1=iota_pc_f[:],
                scalar2=None, op0=ALU.is_equal,
            )

            ones_f = cpool.tile([128, 128], F32, tag="ones_f")
            nc.vector.memset(ones_f[:], 1.0)
            ones_h = cpool.tile([128, 128], F16, tag="ones_h")
            nc.vector.memset(ones_h[:], 1.0)
            mv_l1 = cpool.tile([128, 8], F32, tag="mv_l1")
            nc.vector.memset(mv_l1[:], MASK_VALUE)
            eps128 = cpool.tile([128, 1], F32, tag="eps128")
            nc.vector.memset(eps128[:], EPS)

            # block-diagonal spread mask: bd8[j', (p, j)] = (j == j')
            bd8 = cpool.tile([8, N], F32, tag="bd8")
            nc.gpsimd.affine_select(
                out=bd8[:].rearrange("q (p j) -> q p j", p=128, j=8),
                in_=ones_f[0:8, 0:1].rearrange("q (p u) -> q p u", u=1).to_broadcast(
                    [8, 128, 8]
                ),
                pattern=[[0, 128], [1, 8]],
                compare_op=ALU.is_equal,
                fill=0.0,
                base=0,
                channel_multiplier=-1,
            )

            # hoff24[p, h*2+c] = 1024*h  (for gather indices)
            hoff24 = cpool.tile([128, 24], I32, tag="hoff24")
            nc.gpsimd.iota(hoff24[:], pattern=[[N, 12], [0, 2]], channel_multiplier=0)
            hoff24_f = cpool.tile([128, 24], F32, tag="hoff24_f")
            nc.vector.tensor_copy(hoff24_f[:], hoff24[:])

            # ---------------- value norms (squares on ACT, reduce on DVE) ---
            CW = HC * 8 * D  # cols per chunk in v_sb
            nsq = pool.tile([128, H * 8], F32, tag="nsq")
            nrm = pool.tile([128, H * 8], F32, tag="nrm")
            prod = pool.tile([128, H * 8], F32, tag="prod")
            s_l1 = pool.tile([128, 8], F32, tag="s_l1")
            for m in range(VC):
                cols = slice(m * HC * 8, (m + 1) * HC * 8)
                vsq = pool.tile([128, CW], F32, tag=f"vsq{m % 3}")
                nc.scalar.activation(
                    out=vsq[:], in_=v_sb[:, m * CW : (m + 1) * CW], func=ACTF.Square
                )
                nc.vector.tensor_reduce(
                    out=nsq[:, cols],
                    in_=vsq[:].rearrange("p (hj d) -> p hj d", d=D),
                    axis=mybir.AxisListType.X,
                    op=ALU.add,
                )
                nc.scalar.activation(
                    out=nrm[:, cols], in_=nsq[:, cols], func=ACTF.Sqrt
                )
                nc.vector.tensor_tensor(
                    out=prod[:, cols], in0=ca[:, cols], in1=nrm[:, cols], op=ALU.mult
                )
                sm = pool.tile([128, 8], F32, tag=f"sm{m % 2}")
                nc.vector.tensor_reduce(
                    out=sm[:],
                    in_=prod[:, cols].rearrange("p (h j) -> p j h", h=HC),
                    axis=mybir.AxisListType.X,
                    op=ALU.add,
                )
                if m == 0:
                    nc.vector.tensor_copy(s_l1[:], sm[:])
                else:
                    nc.vector.tensor_tensor(
                        out=s_l1[:], in0=s_l1[:], in1=sm[:], op=ALU.add
                    )
            nc.vector.memset(s_l1[0:1, 0:1], 0.0)  # exclude cls token

            s_red = pool.tile([128, 1], F32, tag="s_red")
            nc.vector.tensor_reduce(
                out=s_red[:], in_=s_l1[:], axis=mybir.AxisListType.X, op=ALU.add
            )
            S_ps = ppoolB.tile([1, 1], F32, tag="psB")
            nc.tensor.matmul(
                out=S_ps[:], lhsT=s_red[:], rhs=ones_f[:, 0:1], start=True, stop=True
            )
            # ln(s/(S+eps) + eps) = ln(s + eps*(S+eps)) - ln(S+eps); the
            # constant shift is argmax-invariant, so only the bias matters.
            epsSe = pool.tile([1, 1], F32, tag="epsSe")
            nc.vector.tensor_scalar(
                out=epsSe[:], in0=S_ps[:], scalar1=EPS, scalar2=EPS,
                op0=ALU.add, op1=ALU.mult,
            )
            eb_ps = ppoolB.tile([128, 1], F32, tag="psB")
            nc.tensor.matmul(
                out=eb_ps[:], lhsT=ones_f[0:1, :], rhs=epsSe[:], start=True, stop=True
            )
            epsSe_bc = pool.tile([128, 1], F32, tag="epsSe_bc")
            nc.vector.tensor_copy(epsSe_bc[:], eb_ps[:])

            logits_l1 = pool.tile([128, 8], F32, tag="logits_l1")
            nc.scalar.activation(
                out=logits_l1[:], in_=s_l1[:], func=ACTF.Ln,
                bias=epsSe_bc[:], scale=1.0,
            )
            logits_m = pool.tile([128, 8], F32, tag="logits_m")
            nc.vector.select(
                out=logits_m[:], mask=mask_l1[:], on_true=logits_l1[:],
                on_false=mv_l1[:],
            )
            nc.vector.memset(logits_m[0:1, 0:1], NEG_BIG)  # kill cls slot

            # L1 -> token-linear row block: PE transpose then block-diag spread
            lt_ps = ppoolB.tile([8, 128], F32, tag="psB")
            nc.tensor.transpose(out=lt_ps[:], in_=logits_m[:], identity=ident[:])
            lt_sb = pool.tile([8, 128], F32, tag="lt_sb")
            nc.vector.tensor_copy(lt_sb[:], lt_ps[:])
            rhs_bd = pool.tile([8, N], F32, tag="rhs_bd")
            nc.vector.tensor_tensor(
                out=rhs_bd[:],
                in0=lt_sb[:].rearrange("q (p u) -> q p u", u=1).to_broadcast(
                    [8, 128, 8]
                ),
                in1=bd8[:].rearrange("q (p j) -> q p j", p=128, j=8),
                op=ALU.mult,
            )

            # broadcast logits over 128 partitions: lb[m, t] = logits[t]
            lb_psum = []
            for half in range(2):
                lb = ppoolA.tile([128, 512], F32, tag="psA")
                nc.tensor.matmul(
                    out=lb[:],
                    lhsT=ones_f[0:8, :],
                    rhs=rhs_bd[:, half * 512 : (half + 1) * 512],
                    start=True,
                    stop=True,
                )
                lb_psum.append(lb)

            # ---------------- gumbel argmax ----------------
            sampled_i = pool.tile([128, 2], I32, tag="sampled_i")
            for k2 in range(2):
                x = pool.tile([128, N], F32, tag=f"x{k2}")
                for half in range(2):
                    cols = slice(half * 512, (half + 1) * 512)
                    nc.vector.tensor_tensor(
                        out=x[:, cols],
                        in0=g_sb[k2][:, cols],
                        in1=lb_psum[half][:],
                        op=ALU.add,
                    )
                mx8 = pool.tile([128, 8], F32, tag=f"mx8_{k2}")
                nc.vector.max(mx8[:], x[:])
                mi8 = pool.tile([128, 8], U32, tag=f"mi8_{k2}")
                nc.vector.max_index(mi8[:], mx8[:], x[:])
                nc.vector.tensor_copy(sampled_i[:, k2 : k2 + 1], mi8[:, 0:1])

            # ---------------- histogram / presence ----------------
            sdiv_i = pool.tile([128, 2], I32, tag="sdiv_i")
            nc.vector.tensor_scalar(
                out=sdiv_i[:], in0=sampled_i[:], scalar1=7, scalar2=None,
                op0=ALU.arith_shift_right,
            )
            smod_i = pool.tile([128, 2], I32, tag="smod_i")
            nc.vector.tensor_scalar(
                out=smod_i[:], in0=sampled_i[:], scalar1=127, scalar2=None,
                op0=ALU.bitwise_and,
            )
            sdiv = pool.tile([128, 2], F32, tag="sdiv")
            nc.vector.tensor_copy(sdiv[:], sdiv_i[:])
            smod = pool.tile([128, 2], F32, tag="smod")
            nc.vector.tensor_copy(smod[:], smod_i[:])

            count_ps = ppoolB.tile([128, 8], F32, tag="psB")
            for k2 in range(2):
                p_eq = pool.tile([128, 128], F16, tag=f"p_eq{k2}")
                nc.vector.tensor_scalar(
                    out=p_eq[:], in0=iota_p_row[:], scalar1=smod[:, k2 : k2 + 1],
                    scalar2=None, op0=ALU.is_equal,
                )
                c_eq = pool.tile([128, 8], F16, tag=f"c_eq{k2}")
                nc.vector.tensor_scalar(
                    out=c_eq[:], in0=iota_c_row[:], scalar1=sdiv[:, k2 : k2 + 1],
                    scalar2=None, op0=ALU.is_equal,
                )
                nc.tensor.matmul(
                    out=count_ps[:], lhsT=p_eq[:], rhs=c_eq[:],
                    start=(k2 == 0), stop=(k2 == 1),
                )
            present = pool.tile([128, 8], F16, tag="present")
            nc.vector.tensor_scalar(
                out=present[:], in0=count_ps[:], scalar1=1.0, scalar2=None,
                op0=ALU.min,
            )

            # ---------------- rank = inclusive prefix sum ----------------
            rank_ps = ppoolB.tile([128, 8], F32, tag="psB")
            nc.tensor.matmul(
                out=rank_ps[:], lhsT=tri_h[:], rhs=present[:], start=True, stop=False
            )
            for c in range(7):
                nc.tensor.matmul(
                    out=rank_ps[:, c + 1 : 8],
                    lhsT=ones_h[:],
                    rhs=present[:, c : c + 1].to_broadcast([128, 7 - c]),
                    start=False,
                    stop=(c == 6),
                )
            rank = pool.tile([128, 8], F32, tag="rank")
            nc.vector.tensor_copy(rank[:], rank_ps[:])

            # ---------------- compact unique ids (direct pm layout) -------
            # uniq_pm[p, c] = token with rank == 128c + p, via separable
            # equality (rank mod 128 == p) x (rank div 128 == c) matmuls.
            w_cm = pool.tile([128, 8], F16, tag="w_cm")
            nc.vector.tensor_tensor(
                out=w_cm[:], in0=iota_tok_h[:], in1=present[:], op=ALU.mult
            )
            rc1 = pool.tile([128, 8], F32, tag="rc1")
            nc.vector.tensor_scalar(
                out=rc1[:], in0=rank[:], scalar1=128.0, scalar2=None, op0=ALU.is_ge
            )
            rc2 = pool.tile([128, 8], F32, tag="rc2")
            nc.vector.tensor_scalar(
                out=rc2[:], in0=rank[:], scalar1=256.0, scalar2=None, op0=ALU.is_ge
            )
            rdiv8 = pool.tile([128, 8], F32, tag="rdiv8")
            nc.vector.tensor_tensor(out=rdiv8[:], in0=rc1[:], in1=rc2[:], op=ALU.add)
            rmul = pool.tile([128, 8], F32, tag="rmul")
            nc.vector.tensor_scalar(
                out=rmul[:], in0=rdiv8[:], scalar1=-128.0, scalar2=None, op0=ALU.mult
            )
            rmod8 = pool.tile([128, 8], F32, tag="rmod8")
            nc.vector.tensor_tensor(out=rmod8[:], in0=rank[:], in1=rmul[:], op=ALU.add)

            peq_all = pool.tile([128, 8 * 128], F16, tag="peq_all")
            nc.vector.tensor_tensor(
                out=peq_all[:].rearrange("p (c q) -> p c q", c=8),
                in0=iota_p_row[:].rearrange("p (u q) -> p u q", u=1).to_broadcast(
                    [128, 8, 128]
                ),
                in1=rmod8[:].rearrange("p (c u) -> p c u", u=1).to_broadcast(
                    [128, 8, 128]
                ),
                op=ALU.is_equal,
            )
            ceq_all = pool.tile([128, 8 * 3], F16, tag="ceq_all")
            nc.vector.tensor_tensor(
                out=ceq_all[:].rearrange("p (c u) -> p c u", c=8),
                in0=iota_c_row[:, 0:3].rearrange("p (u v) -> p u v", u=1).to_broadcast(
                    [128, 8, 3]
                ),
                in1=rdiv8[:].rearrange("p (c u) -> p c u", u=1).to_broadcast(
                    [128, 8, 3]
                ),
                op=ALU.is_equal,
            )
            wceq_all = pool.tile([128, 8 * 3], F16, tag="wceq_all")
            nc.vector.tensor_tensor(
                out=wceq_all[:].rearrange("p (c u) -> p c u", c=8),
                in0=ceq_all[:].rearrange("p (c u) -> p c u", c=8),
                in1=w_cm[:].rearrange("p (c u) -> p c u", u=1).to_broadcast(
                    [128, 8, 3]
                ),
                op=ALU.mult,
            )

            uniqpm_ps = ppoolB.tile([128, 3], F32, tag="psB")
            for c in range(8):
                nc.tensor.matmul(
                    out=uniqpm_ps[:],
                    lhsT=peq_all[:, c * 128 : (c + 1) * 128],
                    rhs=wceq_all[:, c * 3 : (c + 1) * 3],
                    start=(c == 0),
                    stop=(c == 7),
                )
            uniq_pm3 = pool.tile([128, 3], F32, tag="uniq_pm3")
            nc.vector.tensor_copy(uniq_pm3[:], uniqpm_ps[:])

            # idx_pm24[p, h*2+c] = uniq[c*128+p] + 1024*h
            idx_pm24 = pool.tile([128, 24], I32, tag="idx_pm24")
            nc.vector.tensor_tensor(
                out=idx_pm24[:],
                in0=uniq_pm3[:, 0:2].rearrange("p (u c) -> p u c", u=1).to_broadcast(
                    [128, 12, 2]
                ),
                in1=hoff24_f[:].rearrange("p (h c) -> p h c", h=12),
                op=ALU.add,
            )

            # last slot (s=256) for all heads: idx12B[h] = uniq[256] + 1024*h
            idxB_ps = ppoolB.tile([12, 1], F32, tag="psB")
            nc.tensor.matmul(
                out=idxB_ps[:], lhsT=ones_f[0:1, 0:12],
                rhs=uniq_pm3[0:1, 2:3], start=True, stop=True,
            )
            idx12B = pool.tile([12, 1], I32, tag="idx12B")
            nc.vector.tensor_scalar(
                out=idx12B[:], in0=idxB_ps[:], scalar1=iota_h_f[:], scalar2=None,
                op0=ALU.add,
            )

            # outputs uniq/mask via PE transpose of uniq_pm3 (off critical path)
            uniqT_ps = ppoolB.tile([3, 128], F32, tag="psB")
            nc.tensor.transpose(
                out=uniqT_ps[:], in_=uniq_pm3[:], identity=ident[:]
            )
            uniqT = pool.tile([3, 128], F32, tag="uniqT")
            nc.vector.tensor_copy(uniqT[:], uniqT_ps[:])
            uniqT_i32 = pool.tile([3, 128], I32, tag="uniqT_i32")
            nc.vector.tensor_copy(uniqT_i32[:], uniqT[:])
            nc.sync.dma_start(
                out=out_uniq[0:256].rearrange("(c p) -> c p", c=2),
                in_=uniqT_i32[0:2, :],
            )
            nc.sync.dma_start(out=out_uniq[None, 256:257], in_=uniqT_i32[2:3, 0:1])

            maskT = pool.tile([3, 128], U8, tag="maskT")
            nc.vector.tensor_scalar(
                out=maskT[:], in0=uniqT[:], scalar1=0.0, scalar2=None, op0=ALU.is_gt
            )
            nc.vector.memset(maskT[0:1, 0:1], 1)
            nc.sync.dma_start(
                out=out_mask[0:256].rearrange("(c p) -> c p", c=2),
                in_=maskT[0:2, :],
            )
            nc.sync.dma_start(out=out_mask[None, 256:257], in_=maskT[2:3, 0:1])

            # ---------------- gather attention rows ----------------
            # final slot (s = 256) for all heads, issued first
            gbufB = pool.tile([12, N], F32, tag="gbufB")
            nc.gpsimd.indirect_dma_start(
                out=gbufB[:],
                out_offset=None,
                in_=attn_in[:],
                in_offset=bass.IndirectOffsetOnAxis(ap=idx12B[:], axis=0),
            )
            nc.sync.dma_start(out=out_attn[:, 256 * N :], in_=gbufB[:])
            for h in range(H):
                for c in range(2):
                    gbuf = gpool.tile([128, N], F32, tag="gbuf")
                    nc.gpsimd.indirect_dma_start(
                        out=gbuf[:],
                        out_offset=None,
                        in_=attn_in[:],
                        in_offset=bass.IndirectOffsetOnAxis(
                            ap=idx_pm24[:, h * 2 + c : h * 2 + c + 1], axis=0
                        ),
                    )
                    nc.sync.dma_start(
                        out=out_attn[
                            h, c * 128 * N : (c + 1) * 128 * N
                        ].rearrange("(p x) -> p x", p=128),
                        in_=gbuf[:],
                    )

    if waitsplit:
        split_multi_waits(nc)
    return nc


_NC_CACHE = None


def _get_nc():
    global _NC_CACHE
    if _NC_CACHE is None:
        _NC_CACHE = build_nc()
    return _NC_CACHE


def make_in_maps(attn, value, mask, gumbel):
    in_maps = []
    for b in range(B):
        in_maps.append(
            {
                "attn": np.ascontiguousarray(attn[b].reshape(H * N, N), np.float32),
                "value": np.ascontiguousarray(value[b].reshape(H * N, D), np.float32),
                "mask": np.ascontiguousarray(mask[b]).view(np.uint8),
                "gumbel": np.ascontiguousarray(gumbel[b], np.float32),
            }
        )
    return in_maps


TRACE = False
LAST_RESULT = None


def kernel(attn, value, mask, gumbel):
    global LAST_RESULT
    from concourse.bass_utils import run_bass_kernel_spmd

    attn = np.asarray(attn)
    value = np.asarray(value)
    mask = np.asarray(mask)
    gumbel = np.asarray(gumbel)

    nc = _get_nc()
    in_maps = make_in_maps(attn, value, mask, gumbel)
    res = run_bass_kernel_spmd(nc, in_maps, core_ids=list(range(B)), trace=TRACE)
    LAST_RESULT = res

    new_attn = np.stack(
        [res.results[b]["out_attn"].reshape(H, S, N) for b in range(B)]
    ).astype(np.float32)
    uniq = np.stack([res.results[b]["out_uniq"] for b in range(B)]).astype(np.int32)
    new_mask = np.stack(
        [res.results[b]["out_mask"].astype(bool) for b in range(B)]
    )
    return new_attn, new_mask, uniq


# revision 25
# speedup vs baseline: 1.0992x; 1.0992x over previous
"""AdaptiveTokenSampling kernel for 8 TRN2 NeuronCores.

Data-parallel over batch: core i handles batch element i end-to-end
(scoring, gumbel top-k argmax, dedup+compact, attention-row gather).

Pipeline per core (b = batch element):
  s[t]     = sum_h attn[b,h,0,t] * ||value[b,h,t,:]||        (t = 1..1023)
  logits   = log(s / (sum_t s + eps) + eps), masked
  sampled1..256 = argmax_t(logits[t] + gumbel[b,k,t-1])      (256 draws)
  uniq     = sorted unique sampled ids, 0-padded, cls(0) prepended
  new_attn[h,s,:] = attn[b,h,uniq[s],:]

Dedup is sort-free: a histogram of sampled ids is built with two
separable-equality matmuls, ranks come from a triangular-matrix prefix-sum
matmul, and the compacted ids from an equality matmul against the ranks.

Layouts: "L1" token layout t = 8p + j ([128, H*8] tiles, DMA-friendly),
"chunk" layout t = 128c + p ([128, 8] tiles, matmul-friendly); the one
L1->row hop is a PE transpose + affine_select block-diagonal broadcast.
"""

import sys

for p in ("/opt/trn_rl_repo", "/root/.axon_site/_ro/trn_rl_repo"):
    if p not in sys.path:
        sys.path.insert(0, p)

import numpy as np

import concourse.bass as bass
import concourse.mybir as mybir
from concourse.tile import TileContext

B, H, N, D = 8, 12, 1024, 64
K = 256          # number of gumbel draws
S = K + 1        # output slots (cls prepended)
EPS = 1e-6
MASK_VALUE = -np.finfo(np.float32).max / 2
NEG_BIG = -1.0e30

F32 = mybir.dt.float32
F16 = mybir.dt.float16
I32 = mybir.dt.int32
U32 = mybir.dt.uint32
U8 = mybir.dt.uint8
ALU = mybir.AluOpType
ACTF = mybir.ActivationFunctionType


def split_multi_waits(nc, max_waits=1):
    """Walrus codegen rejects instructions with several sem waits; hoist
    extra waits onto same-engine NoOps inserted immediately before."""
    for fn in nc.m.functions:
        for blk in fn.blocks:
            new_insts = []
            for inst in blk.instructions:
                si = getattr(inst, "sync_info", None)
                if si is not None and si.on_wait and len(si.on_wait) > max_waits:
                    waits = list(si.on_wait)
                    for j, w in enumerate(waits[:-max_waits]):
                        new_insts.append(
                            mybir.InstNoOp(
                                name=f"{inst.name}_wsplit{j}",
                                sync_info=mybir.SyncInfo(on_wait=[w], on_update=[]),
                                bass_nofuse=True,
                                engine=inst.engine,
                            )
                        )
                    inst.sync_info = mybir.SyncInfo(
                        on_wait=waits[-max_waits:], on_update=si.on_update
                    )
                new_insts.append(inst)
            blk.instructions[:] = new_insts


def build_nc(waitsplit=True):
    nc = bass.Bass()

    attn_in = nc.declare_dram_parameter("attn", [H * N, N], F32, isOutput=False)
    value_in = nc.declare_dram_parameter("value", [H * N, D], F32, isOutput=False)
    mask_in = nc.declare_dram_parameter("mask", [N], U8, isOutput=False)
    gumbel_in = nc.declare_dram_parameter("gumbel", [K, N - 1], F32, isOutput=False)

    out_attn = nc.declare_dram_parameter("out_attn", [H, S * N], F32, isOutput=True)
    out_uniq = nc.declare_dram_parameter("out_uniq", [S], I32, isOutput=True)
    out_mask = nc.declare_dram_parameter("out_mask", [S], U8, isOutput=True)

    VC = 6               # value DMA/norm pipeline chunks
    HC = H // VC         # heads per chunk

    with TileContext(nc) as tc:
        with (
            tc.tile_pool(name="const", bufs=1) as cpool,
            tc.tile_pool(name="sbuf", bufs=1) as pool,
            tc.tile_pool(name="gather", bufs=8) as gpool,
            tc.tile_pool(name="psumA", bufs=2, space="PSUM") as ppoolA,
            tc.tile_pool(name="psumB", bufs=3, space="PSUM") as ppoolB,
        ):
            # ---------------- input DMAs first ----------------
            # value in "L1" layout: v_sb[p, h*512 + j*64 + d] = value[h, 8p+j, d]
            v_sb = pool.tile([128, H * 8 * D], F32, tag="v_sb")
            v_dst = v_sb[:].rearrange("p (h j d) -> p h j d", h=H, j=8, d=D)
            v_src = value_in[:].rearrange("(h p j) d -> p h j d", h=H, p=128, j=8)
            for m in range(VC):
                nc.sync.dma_start(
                    out=v_dst[:, m * HC : (m + 1) * HC],
                    in_=v_src[:, m * HC : (m + 1) * HC],
                )

            # gumbel tiles: g_sb[k2][p, t] = gumbel[k2*128+p, t-1]; col 0 = 0
            g_sb = []
            for k2 in range(2):
                g = pool.tile([128, N], F32, tag=f"g_sb{k2}")
                nc.sync.dma_start(
                    out=g[:, 1:N], in_=gumbel_in[k2 * 128 : (k2 + 1) * 128, :]
                )
                nc.vector.memset(g[:, 0:1], 0.0)
                g_sb.append(g)

            # cls attention rows in L1: ca[p, h*8+j] = attn[h*N, 8p+j]
            ca = pool.tile([128, H * 8], F32, tag="ca")
            nc.sync.dma_start(
                out=ca[:].rearrange("p (h j) -> p h j", h=H),
                in_=attn_in[:].rearrange("(h n) (p j) -> n p h j", h=H, p=128)[0],
            )

            # mask in L1: mask_l1[p, j] = mask[8p+j]
            mask_l1 = pool.tile([128, 8], U8, tag="mask_l1")
            nc.sync.dma_start(
                out=mask_l1[:], in_=mask_in[:].rearrange("(p j) -> p j", p=128)
            )

            # ---------------- constants ----------------
            iota_p_i = cpool.tile([128, 128], I32, tag="iota_p_i")
            nc.gpsimd.iota(iota_p_i[:], pattern=[[1, 128]], channel_multiplier=0)
            iota_p_row = cpool.tile([128, 128], F32, tag="iota_p_row")
            nc.vector.tensor_copy(iota_p_row[:], iota_p_i[:])

            iota_c_i = cpool.tile([128, 8], I32, tag="iota_c_i")
            nc.gpsimd.iota(iota_c_i[:], pattern=[[1, 8]], channel_multiplier=0)
            iota_c_row = cpool.tile([128, 8], F32, tag="iota_c_row")
            nc.vector.tensor_copy(iota_c_row[:], iota_c_i[:])

            iota_tok_i = cpool.tile([128, 8], I32, tag="iota_tok_i")
            nc.gpsimd.iota(iota_tok_i[:], pattern=[[128, 8]], channel_multiplier=1)
            iota_tok_h = cpool.tile([128, 8], F16, tag="iota_tok_h")
            nc.vector.tensor_copy(iota_tok_h[:], iota_tok_i[:])

            tri_i = cpool.tile([128, 128], I32, tag="tri_i")
            nc.gpsimd.iota(tri_i[:], pattern=[[1, 128]], channel_multiplier=-1)
            tri_raw = cpool.tile([128, 128], F32, tag="tri_raw")
            nc.vector.tensor_copy(tri_raw[:], tri_i[:])
            tri_h = cpool.tile([128, 128], F16, tag="tri_h")
            nc.vector.tensor_scalar(
                out=tri_h[:], in0=tri_raw[:], scalar1=0.0, scalar2=None, op0=ALU.is_ge
            )

            iota_s_i = cpool.tile([128, S], I32, tag="iota_s_i")
            nc.gpsimd.iota(iota_s_i[:], pattern=[[1, S]], channel_multiplier=0)
            iota_s_f = cpool.tile([128, S], F32, tag="iota_s_f")
            nc.vector.tensor_copy(iota_s_f[:], iota_s_i[:])

            iota_h_i = cpool.tile([12, 1], I32, tag="iota_h_i")
            nc.gpsimd.iota(iota_h_i[:], pattern=[[0, 1]], channel_multiplier=N)
            iota_h_f = cpool.tile([12, 1], F32, tag="iota_h_f")
            nc.vector.tensor_copy(iota_h_f[:], iota_h_i[:])

            iota_pc_i = cpool.tile([128, 1], I32, tag="iota_pc_i")
            nc.gpsimd.iota(iota_pc_i[:], pattern=[[0, 1]], channel_multiplier=1)
            iota_pc_f = cpool.tile([128, 1], F32, tag="iota_pc_f")
            nc.vector.tensor_copy(iota_pc_f[:], iota_pc_i[:])
            ident = cpool.tile([128, 128], F32, tag="ident")
            nc.vector.tensor_scalar(
                out=ident[:], in0=iota_p_row[:], scalar1=iota_pc_f[:],
                scalar2=None, op0=ALU.is_equal,
            )

            ones_f = cpool.tile([128, 128], F32, tag="ones_f")
            nc.vector.memset(ones_f[:], 1.0)
            ones_h = cpool.tile([128, 128], F16, tag="ones_h")
            nc.vector.memset(ones_h[:], 1.0)
            mv_l1 = cpool.tile([128, 8], F32, tag="mv_l1")
            nc.vector.memset(mv_l1[:], MASK_VALUE)
            eps128 = cpool.tile([128, 1], F32, tag="eps128")
            nc.vector.memset(eps128[:], EPS)

            # block-diagonal spread mask: bd8[j', (p, j)] = (j == j')
            bd8 = cpool.tile([8, N], F32, tag="bd8")
            nc.gpsimd.affine_select(
                out=bd8[:].rearrange("q (p j) -> q p j", p=128, j=8),
                in_=ones_f[0:8, 0:1].rearrange("q (p u) -> q p u", u=1).to_broadcast(
                    [8, 128, 8]
                ),
                pattern=[[0, 128], [1, 8]],
                compare_op=ALU.is_equal,
                fill=0.0,
                base=0,
                channel_multiplier=-1,
            )

            # hoff24[p, h*2+c] = 1024*h  (for gather indices)
            hoff24 = cpool.tile([128, 24], I32, tag="hoff24")
            nc.gpsimd.iota(hoff24[:], pattern=[[N, 12], [0, 2]], channel_multiplier=0)
            hoff24_f = cpool.tile([128, 24], F32, tag="hoff24_f")
            nc.vector.tensor_copy(hoff24_f[:], hoff24[:])

            # ---------------- value norms (squares on ACT, reduce on DVE) ---
            CW = HC * 8 * D  # cols per chunk in v_sb
            nsq = pool.tile([128, H * 8], F32, tag="nsq")
            nrm = pool.tile([128, H * 8], F32, tag="nrm")
            for m in range(VC):
                vsq = pool.tile([128, CW], F32, tag=f"vsq{m % 3}")
                nc.scalar.activation(
                    out=vsq[:], in_=v_sb[:, m * CW : (m + 1) * CW], func=ACTF.Square
                )
                nc.vector.tensor_reduce(
                    out=nsq[:, m * HC * 8 : (m + 1) * HC * 8],
                    in_=vsq[:].rearrange("p (hj d) -> p hj d", d=D),
                    axis=mybir.AxisListType.X,
                    op=ALU.add,
                )
            nc.scalar.activation(out=nrm[:], in_=nsq[:], func=ACTF.Sqrt)

            # ---------------- scores + logits (L1 layout) ----------------
            prod = pool.tile([128, H * 8], F32, tag="prod")
            nc.vector.tensor_tensor(out=prod[:], in0=ca[:], in1=nrm[:], op=ALU.mult)
            s_l1 = pool.tile([128, 8], F32, tag="s_l1")
            nc.vector.tensor_reduce(
                out=s_l1[:],
                in_=prod[:].rearrange("p (h j) -> p j h", h=H),
                axis=mybir.AxisListType.X,
                op=ALU.add,
            )
            nc.vector.memset(s_l1[0:1, 0:1], 0.0)  # exclude cls token

            s_red = pool.tile([128, 1], F32, tag="s_red")
            nc.vector.tensor_reduce(
                out=s_red[:], in_=s_l1[:], axis=mybir.AxisListType.X, op=ALU.add
            )
            S_ps = ppoolB.tile([1, 1], F32, tag="psB")
            nc.tensor.matmul(
                out=S_ps[:], lhsT=s_red[:], rhs=ones_f[:, 0:1], start=True, stop=True
            )
            # ln(s/(S+eps) + eps) = ln(s + eps*(S+eps)) - ln(S+eps); the
            # constant shift is argmax-invariant, so only the bias matters.
            epsSe = pool.tile([1, 1], F32, tag="epsSe")
            nc.vector.tensor_scalar(
                out=epsSe[:], in0=S_ps[:], scalar1=EPS, scalar2=EPS,
                op0=ALU.add, op1=ALU.mult,
            )
            eb_ps = ppoolB.tile([128, 1], F32, tag="psB")
            nc.tensor.matmul(
                out=eb_ps[:], lhsT=ones_f[0:1, :], rhs=epsSe[:], start=True, stop=True
            )
            epsSe_bc = pool.tile([128, 1], F32, tag="epsSe_bc")
            nc.vector.tensor_copy(epsSe_bc[:], eb_ps[:])

            logits_l1 = pool.tile([128, 8], F32, tag="logits_l1")
            nc.scalar.activation(
                out=logits_l1[:], in_=s_l1[:], func=ACTF.Ln,
                bias=epsSe_bc[:], scale=1.0,
            )
            logits_m = pool.tile([128, 8], F32, tag="logits_m")
            nc.vector.select(
                out=logits_m[:], mask=mask_l1[:], on_true=logits_l1[:],
                on_false=mv_l1[:],
            )
            nc.vector.memset(logits_m[0:1, 0:1], NEG_BIG)  # kill cls slot

            # L1 -> token-linear row block: PE transpose then block-diag spread
            lt_ps = ppoolB.tile([8, 128], F32, tag="psB")
            nc.tensor.transpose(out=lt_ps[:], in_=logits_m[:], identity=ident[:])
            lt_sb = pool.tile([8, 128], F32, tag="lt_sb")
            nc.vector.tensor_copy(lt_sb[:], lt_ps[:])
            rhs_bd = pool.tile([8, N], F32, tag="rhs_bd")
            nc.vector.tensor_tensor(
                out=rhs_bd[:],
                in0=lt_sb[:].rearrange("q (p u) -> q p u", u=1).to_broadcast(
                    [8, 128, 8]
                ),
                in1=bd8[:].rearrange("q (p j) -> q p j", p=128, j=8),
                op=ALU.mult,
            )

            # broadcast logits over 128 partitions: lb[m, t] = logits[t]
            lb_psum = []
            for half in range(2):
                lb = ppoolA.tile([128, 512], F32, tag="psA")
                nc.tensor.matmul(
                    out=lb[:],
                    lhsT=ones_f[0:8, :],
                    rhs=rhs_bd[:, half * 512 : (half + 1) * 512],
                    start=True,
                    stop=True,
                )
                lb_psum.append(lb)

            # ---------------- gumbel argmax ----------------
            sampled_i = pool.tile([128, 2], I32, tag="sampled_i")
            for k2 in range(2):
                x = pool.tile([128, N], F32, tag=f"x{k2}")
                for half in range(2):
                    cols = slice(half * 512, (half + 1) * 512)
                    nc.vector.tensor_tensor(
                        out=x[:, cols],
                        in0=g_sb[k2][:, cols],
                        in1=lb_psum[half][:],
                        op=ALU.add,
                    )
                mx8 = pool.tile([128, 8], F32, tag=f"mx8_{k2}")
                nc.vector.max(mx8[:], x[:])
                mi8 = pool.tile([128, 8], U32, tag=f"mi8_{k2}")
                nc.vector.max_index(mi8[:], mx8[:], x[:])
                nc.vector.tensor_copy(sampled_i[:, k2 : k2 + 1], mi8[:, 0:1])

            # ---------------- histogram / presence ----------------
            sdiv_i = pool.tile([128, 2], I32, tag="sdiv_i")
            nc.vector.tensor_scalar(
                out=sdiv_i[:], in0=sampled_i[:], scalar1=7, scalar2=None,
                op0=ALU.arith_shift_right,
            )
            smod_i = pool.tile([128, 2], I32, tag="smod_i")
            nc.vector.tensor_scalar(
                out=smod_i[:], in0=sampled_i[:], scalar1=127, scalar2=None,
                op0=ALU.bitwise_and,
            )
            sdiv = pool.tile([128, 2], F32, tag="sdiv")
            nc.vector.tensor_copy(sdiv[:], sdiv_i[:])
            smod = pool.tile([128, 2], F32, tag="smod")
            nc.vector.tensor_copy(smod[:], smod_i[:])

            count_ps = ppoolB.tile([128, 8], F32, tag="psB")
            for k2 in range(2):
                p_eq = pool.tile([128, 128], F16, tag=f"p_eq{k2}")
                nc.vector.tensor_scalar(
                    out=p_eq[:], in0=iota_p_row[:], scalar1=smod[:, k2 : k2 + 1],
                    scalar2=None, op0=ALU.is_equal,
                )
                c_eq = pool.tile([128, 8], F16, tag=f"c_eq{k2}")
                nc.vector.tensor_scalar(
                    out=c_eq[:], in0=iota_c_row[:], scalar1=sdiv[:, k2 : k2 + 1],
                    scalar2=None, op0=ALU.is_equal,
                )
                nc.tensor.matmul(
                    out=count_ps[:], lhsT=p_eq[:], rhs=c_eq[:],
                    start=(k2 == 0), stop=(k2 == 1),
                )
            present = pool.tile([128, 8], F16, tag="present")
            nc.vector.tensor_scalar(
                out=present[:], in0=count_ps[:], scalar1=1.0, scalar2=None,
                op0=ALU.min,
            )

            # ---------------- rank = inclusive prefix sum ----------------
            rank_ps = ppoolB.tile([128, 8], F32, tag="psB")
            nc.tensor.matmul(
                out=rank_ps[:], lhsT=tri_h[:], rhs=present[:], start=True, stop=False
            )
            for c in range(7):
                nc.tensor.matmul(
                    out=rank_ps[:, c + 1 : 8],
                    lhsT=ones_h[:],
                    rhs=present[:, c : c + 1].to_broadcast([128, 7 - c]),
                    start=False,
                    stop=(c == 6),
                )
            rank = pool.tile([128, 8], F32, tag="rank")
            nc.vector.tensor_copy(rank[:], rank_ps[:])

            # ---------------- compact unique ids (direct pm layout) -------
            # uniq_pm[p, c] = token with rank == 128c + p, via separable
            # equality (rank mod 128 == p) x (rank div 128 == c) matmuls.
            w_cm = pool.tile([128, 8], F16, tag="w_cm")
            nc.vector.tensor_tensor(
                out=w_cm[:], in0=iota_tok_h[:], in1=present[:], op=ALU.mult
            )
            rc1 = pool.tile([128, 8], F32, tag="rc1")
            nc.vector.tensor_scalar(
                out=rc1[:], in0=rank[:], scalar1=128.0, scalar2=None, op0=ALU.is_ge
            )
            rc2 = pool.tile([128, 8], F32, tag="rc2")
            nc.vector.tensor_scalar(
                out=rc2[:], in0=rank[:], scalar1=256.0, scalar2=None, op0=ALU.is_ge
            )
            rdiv8 = pool.tile([128, 8], F32, tag="rdiv8")
            nc.vector.tensor_tensor(out=rdiv8[:], in0=rc1[:], in1=rc2[:], op=ALU.add)
            rmul = pool.tile([128, 8], F32, tag="rmul")
            nc.vector.tensor_scalar(
                out=rmul[:], in0=rdiv8[:], scalar1=-128.0, scalar2=None, op0=ALU.mult
            )
            rmod8 = pool.tile([128, 8], F32, tag="rmod8")
            nc.vector.tensor_tensor(out=rmod8[:], in0=rank[:], in1=rmul[:], op=ALU.add)

            peq_all = pool.tile([128, 8 * 128], F16, tag="peq_all")
            nc.vector.tensor_tensor(
                out=peq_all[:].rearrange("p (c q) -> p c q", c=8),
                in0=iota_p_row[:].rearrange("p (u q) -> p u q", u=1).to_broadcast(
                    [128, 8, 128]
                ),
                in1=rmod8[:].rearrange("p (c u) -> p c u", u=1).to_broadcast(
                    [128, 8, 128]
                ),
                op=ALU.is_equal,
            )
            ceq_all = pool.tile([128, 8 * 3], F16, tag="ceq_all")
            nc.vector.tensor_tensor(
                out=ceq_all[:].rearrange("p (c u) -> p c u", c=8),
                in0=iota_c_row[:, 0:3].rearrange("p (u v) -> p u v", u=1).to_broadcast(
                    [128, 8, 3]
                ),
                in1=rdiv8[:].rearrange("p (c u) -> p c u", u=1).to_broadcast(
                    [128, 8, 3]
                ),
                op=ALU.is_equal,
            )
            wceq_all = pool.tile([128, 8 * 3], F16, tag="wceq_all")
            nc.vector.tensor_tensor(
                out=wceq_all[:].rearrange("p (c u) -> p c u", c=8),
                in0=ceq_all[:].rearrange("p (c u) -> p c u", c=8),
                in1=w_cm[:].rearrange("p (c u) -> p c u", u=1).to_broadcast(
                    [128, 8, 3]
                ),
                op=ALU.mult,
            )

            uniqpm_ps = ppoolB.tile([128, 3], F32, tag="psB")
            for c in range(8):
                nc.tensor.matmul(
                    out=uniqpm_ps[:],
                    lhsT=peq_all[:, c * 128 : (c + 1) * 128],
                    rhs=wceq_all[:, c * 3 : (c + 1) * 3],
                    start=(c == 0),
                    stop=(c == 7),
                )
            uniq_pm3 = pool.tile([128, 3], F32, tag="uniq_pm3")
            nc.vector.tensor_copy(uniq_pm3[:], uniqpm_ps[:])

            # idx_pm24[p, h*2+c] = uniq[c*128+p] + 1024*h
            idx_pm24 = pool.tile([128, 24], I32, tag="idx_pm24")
            nc.vector.tensor_tensor(
                out=idx_pm24[:],
                in0=uniq_pm3[:, 0:2].rearrange("p (u c) -> p u c", u=1).to_broadcast(
                    [128, 12, 2]
                ),
                in1=hoff24_f[:].rearrange("p (h c) -> p h c", h=12),
                op=ALU.add,
            )

            # last slot (s=256) for all heads: idx12B[h] = uniq[256] + 1024*h
            idxB_ps = ppoolB.tile([12, 1], F32, tag="psB")
            nc.tensor.matmul(
                out=idxB_ps[:], lhsT=ones_f[0:1, 0:12],
                rhs=uniq_pm3[0:1, 2:3], start=True, stop=True,
            )
            idx12B = pool.tile([12, 1], I32, tag="idx12B")
            nc.vector.tensor_scalar(
                out=idx12B[:], in0=idxB_ps[:], scalar1=iota_h_f[:], scalar2=None,
                op0=ALU.add,
            )

            # outputs uniq/mask via PE transpose of uniq_pm3 (off critical path)
            uniqT_ps = ppoolB.tile([3, 128], F32, tag="psB")
            nc.tensor.transpose(
                out=uniqT_ps[:], in_=uniq_pm3[:], identity=ident[:]
            )
            uniqT = pool.tile([3, 128], F32, tag="uniqT")
            nc.vector.tensor_copy(uniqT[:], uniqT_ps[:])
            uniqT_i32 = pool.tile([3, 128], I32, tag="uniqT_i32")
            nc.vector.tensor_copy(uniqT_i32[:], uniqT[:])
            nc.sync.dma_start(
                out=out_uniq[0:256].rearrange("(c p) -> c p", c=2),
                in_=uniqT_i32[0:2, :],
            )
            nc.sync.dma_start(out=out_uniq[None, 256:257], in_=uniqT_i32[2:3, 0:1])

            maskT = pool.tile([3, 128], U8, tag="maskT")
            nc.vector.tensor_scalar(
                out=maskT[:], in0=uniqT[:], scalar1=0.0, scalar2=None, op0=ALU.is_gt
            )
            nc.vector.memset(maskT[0:1, 0:1], 1)
            nc.sync.dma_start(
                out=out_mask[0:256].rearrange("(c p) -> c p", c=2),
                in_=maskT[0:2, :],
            )
            nc.sync.dma_start(out=out_mask[None, 256:257], in_=maskT[2:3, 0:1])

            # ---------------- gather attention rows ----------------
            # final slot (s = 256) for all heads, issued first
            gbufB = pool.tile([12, N], F32, tag="gbufB")
            nc.gpsimd.indirect_dma_start(
                out=gbufB[:],
                out_offset=None,
                in_=attn_in[:],
                in_offset=bass.IndirectOffsetOnAxis(ap=idx12B[:], axis=0),
            )
            nc.sync.dma_start(out=out_attn[:, 256 * N :], in_=gbufB[:])
            for h in range(H):
                for c in range(2):
                    gbuf = gpool.tile([128, N], F32, tag="gbuf")
                    nc.gpsimd.indirect_dma_start(
                        out=gbuf[:],
                        out_offset=None,
                        in_=attn_in[:],
                        in_offset=bass.IndirectOffsetOnAxis(
                            ap=idx_pm24[:, h * 2 + c : h * 2 + c + 1], axis=0
                        ),
                    )
                    nc.sync.dma_start(
                        out=out_attn[
                            h, c * 128 * N : (c + 1) * 128 * N
                        ].rearrange("(p x) -> p x", p=128),
                        in_=gbuf[:],
                    )

    if waitsplit:
        split_multi_waits(nc)
    return nc


_NC_CACHE = None


def _get_nc():
    global _NC_CACHE
    if _NC_CACHE is None:
        _NC_CACHE = build_nc()
    return _NC_CACHE


def make_in_maps(attn, value, mask, gumbel):
    in_maps = []
    for b in range(B):
        in_maps.append(
            {
                "attn": np.ascontiguousarray(attn[b].reshape(H * N, N), np.float32),
                "value": np.ascontiguousarray(value[b].reshape(H * N, D), np.float32),
                "mask": np.ascontiguousarray(mask[b]).view(np.uint8),
                "gumbel": np.ascontiguousarray(gumbel[b], np.float32),
            }
        )
    return in_maps


TRACE = False
LAST_RESULT = None


def kernel(attn, value, mask, gumbel):
    global LAST_RESULT
    from concourse.bass_utils import run_bass_kernel_spmd

    attn = np.asarray(attn)
    value = np.asarray(value)
    mask = np.asarray(mask)
    gumbel = np.asarray(gumbel)

    nc = _get_nc()
    in_maps = make_in_maps(attn, value, mask, gumbel)
    res = run_bass_kernel_spmd(nc, in_maps, core_ids=list(range(B)), trace=TRACE)
    LAST_RESULT = res

    new_attn = np.stack(
        [res.results[b]["out_attn"].reshape(H, S, N) for b in range(B)]
    ).astype(np.float32)
    uniq = np.stack([res.results[b]["out_uniq"] for b in range(B)]).astype(np.int32)
    new_mask = np.stack(
        [res.results[b]["out_mask"].astype(bool) for b in range(B)]
    )
    return new_attn, new_mask, uniq
